# revision 8
# baseline (speedup 1.0000x reference)
"""DLRM DotInteraction kernel for Trainium2 (Bass/Tile), 8-core data parallel.

Problem: dense_feature [B=16384, D=128] f32, sparse_stack [S=26, B, D] f32.
cat = [dense; sparse] per sample -> [B, N=27, D]; G_b = cat_b @ cat_b^T;
out = [dense | tril(G_b) (378 vals, row-major incl diag)] -> [B, 506] f32.

Strategy per core (B_c = 2048 samples):
  1. SWDGE DMA-in with f32->f16 cast: natural tiles [128 samples, 27 feat, 128 d].
  2. HWDGE xbar DMA-transpose per (tile, feature): [128 s, 128 d] -> [128 d, 128 s].
  3. TensorE Gram: per 4-sample group, 4 col-tiled matmuls (tile_position (0,32c)),
     K=128 (d), M=N=27, fp32 PSUM accumulate.
  4. DVE copies PSUM -> SBUF "gcol" layout [partition 32c+i, group, j].
  5. 27*4 strided HWDGE DMAs per 128-group chunk write the tril part of the
     output rows; dense columns are a direct DRAM->DRAM strided copy.
"""

import numpy as np

import concourse.bacc as bacc
import concourse.mybir as mybir
import concourse.tile as tile
from concourse import bass_utils

B = 16384
D = 128
S = 26
N = S + 1  # 27
NCORES = 8
BC = B // NCORES  # 2048 samples per core
PT = 128  # samples per sbuf tile
GPR = 16  # 4-sample groups per psum round
TRI = N * (N + 1) // 2  # 378
W = D + TRI  # 506
CHUNK_G = 128  # groups per gcol chunk (= 512 samples)

f32 = mybir.dt.float32
f16 = mybir.dt.float16


def build_kernel(b_core: int = BC):
    nc = bacc.Bacc("TRN2", target_bir_lowering=False, debug=False)
    dense = nc.dram_tensor("dense", [b_core, D], f32, kind="ExternalInput").ap()
    sparse = nc.dram_tensor("sparse", [S, b_core, D], f32, kind="ExternalInput").ap()
    out = nc.dram_tensor("out", [b_core, W], f32, kind="ExternalOutput").ap()

    t_total = b_core // PT
    gpt = PT // 4  # 32 groups per tile
    rpt = gpt // GPR  # 2 psum rounds per tile

    with tile.TileContext(nc) as tc:
        with (
            tc.tile_pool(name="nat", bufs=3) as nat_pool,
            tc.tile_pool(name="xt", bufs=3) as xt_pool,
            tc.tile_pool(name="gcol", bufs=2) as gcol_pool,
            tc.tile_pool(name="row", bufs=3) as row_pool,
            tc.tile_pool(name="psum", bufs=3, space="PSUM") as psum_pool,
        ):
            for t in range(t_total):
                # --- load + cast to fp16, natural layout [s, j, d] ---
                nat = nat_pool.tile([128, N, D], f16)
                nc.gpsimd.dma_start(
                    out=nat[:, 0, :], in_=dense[t * PT : (t + 1) * PT, :]
                )
                nc.gpsimd.dma_start(
                    out=nat[:, 1:N, :],
                    in_=sparse[:, t * PT : (t + 1) * PT, :].rearrange(
                        "s b d -> b s d"
                    ),
                )

                # --- xbar transpose each feature slab: [s, d] -> [d, s] ---
                # 32 slabs: 27 features + 5 zero pad so Gram lhsT can be M=32
                # (all PSUM partitions written -> single-op DVE copy later).
                xt = xt_pool.tile([128, 32, PT], f16)
                nc.vector.memset(xt[:, N:32, :], 0.0)
                for j in range(N):
                    nc.sync.dma_start(out=xt[:, j, :], in_=nat[:, j, :], transpose=True)

                # --- Gram matmuls ---
                # group g holds samples {32c + g}; MM (g, c) writes PSUM
                # partitions [32c, 32c+32), so gcol[32c+i, g, j] = Gram[i, j]
                # of sample 32c + g.
                gcol = gcol_pool.tile([128, gpt, N], f32)
                for r in range(rpt):
                    ps = psum_pool.tile([128, GPR, N], f32)
                    for q in range(GPR):
                        g_local = r * GPR + q
                        for c in range(4):
                            s_local = 32 * c + g_local
                            nc.tensor.matmul(
                                out=ps[32 * c : 32 * c + 32, q, :],
                                lhsT=xt[:, :, s_local],  # [128 d, 32 (27j+5pad)]
                                rhs=xt[:, 0:N, s_local],  # [128 d, 27 j]
                                start=True,
                                stop=True,
                                tile_position=(0, 32 * c),
                            )
                    off = r * GPR
                    nc.vector.tensor_copy(
                        out=gcol[:, off : off + GPR, :], in_=ps[:, :, :]
                    )

                # --- assemble full output rows [128 samples, 506] in SBUF ---
                rowtile = row_pool.tile([128, W], f32)
                nc.sync.dma_start(
                    out=rowtile[:, 0:D], in_=dense[t * PT : (t + 1) * PT, :]
                )
                # sample 32c + g gets Gram row i from gcol partition 32c+i:
                # contiguous 32-partition write, single-partition read.
                for i in range(N):
                    toff = D + i * (i + 1) // 2
                    for c in range(4):
                        eng = nc.scalar if (i + c) % 2 else nc.sync
                        eng.dma_start(
                            out=rowtile[32 * c : 32 * c + 32, toff : toff + i + 1],
                            in_=gcol[32 * c + i : 32 * c + i + 1, :, 0 : i + 1],
                        )
                # final contiguous store of 128 rows
                nc.scalar.dma_start(
                    out=out[t * PT : (t + 1) * PT, :], in_=rowtile[:, :]
                )

    nc.compile()
    return nc


_CACHE: dict = {}


def _get_nc():
    if "nc" not in _CACHE:
        _CACHE["nc"] = build_kernel(BC)
    return _CACHE["nc"]


def kernel(dense_feature, sparse_stack, **run_kwargs):
    dense_feature = np.asarray(dense_feature, dtype=np.float32)
    sparse_stack = np.asarray(sparse_stack, dtype=np.float32)
    assert dense_feature.shape == (B, D)
    assert sparse_stack.shape == (S, B, D)

    nc = _get_nc()
    in_maps = []
    for ci in range(NCORES):
        sl = slice(ci * BC, (ci + 1) * BC)
        in_maps.append(
            {
                "dense": np.ascontiguousarray(dense_feature[sl]),
                "sparse": np.ascontiguousarray(sparse_stack[:, sl, :]),
            }
        )
    res = bass_utils.run_bass_kernel_spmd(
        nc, in_maps, core_ids=list(range(NCORES)), **run_kwargs
    )
    out = np.concatenate([r["out"] for r in res.results], axis=0)
    if run_kwargs:
        _CACHE["last_result"] = res
    return out


# revision 13
# speedup vs baseline: 30170.5958x; 30170.5958x over previous
"""DLRM DotInteraction kernel for Trainium2 (Bass/Tile), 8-core data parallel.

Problem: dense_feature [B=16384, D=128] f32, sparse_stack [S=26, B, D] f32.
cat = [dense; sparse] per sample -> [B, N=27, D]; G_b = cat_b @ cat_b^T;
out = [dense | tril(G_b) (378 vals, row-major incl diag)] -> [B, 506] f32.

Per core (B_c = 2048 samples), pipelined over supertiles of 8x128-sample tiles:
  1. Input f32 -> f16: even tiles via SWDGE cast-DMA; odd tiles via HWDGE f32
     load + ScalarE cast-copy (splits the load across DGE paths).
  2. Transpose [128 s, 128 d] -> [128 d, 128 s] per feature slab on TensorE
     (fp16 transpose-mode matmul vs identity), packed 4 slabs per PSUM bank,
     copied back to SBUF by DVE/ScalarE.
  3. TensorE Gram: group g = samples {32c + g}; 4 col-tiled matmuls per group
     (tile_position (0,32c)), K=128 (d), M=32 (27+5 zero pad), N=27, f32 PSUM.
  4. DVE copies Gram PSUM -> SBUF gcol [partition 32c+i, g, tp, j].
  5. Flatten: 27*4 DMAs per supertile gather Gram rows into output-row tiles
     [sample partition, 506]; dense cols loaded from HBM; one contiguous
     259 KB store per 128-sample tile.
"""

import numpy as np

import concourse.bacc as bacc
import concourse.mybir as mybir
import concourse.tile as tile
from concourse import bass_utils
from concourse.masks import make_identity

B = 16384
D = 128
S = 26
N = S + 1  # 27
NCORES = 8
BC = B // NCORES  # 2048 samples per core
PT = 128  # samples per sbuf tile
GPR = 16  # groups per psum round
TRI = N * (N + 1) // 2  # 378
W = D + TRI  # 506
TPS = 8  # tiles per supertile

f32 = mybir.dt.float32
f16 = mybir.dt.float16


def build_kernel(b_core: int = BC, reps: int = 1):
    nc = bacc.Bacc("TRN2", target_bir_lowering=False, debug=False)
    dense = nc.dram_tensor("dense", [b_core, D], f32, kind="ExternalInput").ap()
    sparse = nc.dram_tensor("sparse", [S, b_core, D], f32, kind="ExternalInput").ap()
    out = nc.dram_tensor("out", [b_core, W], f32, kind="ExternalOutput").ap()

    t_total = b_core // PT
    gpt = PT // 4  # 32 groups per tile
    rpt = gpt // GPR  # psum rounds per tile
    tps = min(TPS, t_total)
    n_super = t_total // tps

    with tile.TileContext(nc) as tc:
        with (
            tc.tile_pool(name="singles", bufs=1) as singles,
            tc.tile_pool(name="nat32", bufs=2) as nat32_pool,
            tc.tile_pool(name="nat", bufs=3) as nat_pool,
            tc.tile_pool(name="xt", bufs=3) as xt_pool,
            tc.tile_pool(name="gcol", bufs=2) as gcol_pool,
            tc.tile_pool(name="row", bufs=2) as row_pool,
            tc.tile_pool(name="psum", bufs=3, space="PSUM") as psum_pool,
            tc.tile_pool(name="psumt", bufs=3, space="PSUM") as psumt_pool,
        ):
            id16 = singles.tile([128, 128], f16, name="id16")
            make_identity(nc, id16)

            for _rep in range(reps):
                for st in range(n_super):
                    # gcol[32c+i, g, tp, j] = Gram[i,j] of sample 32c+g in
                    # tile tp of this supertile. (g, tp) order makes the
                    # flatten read dims mergeable (3-dim DMA AP limit).
                    gcol = gcol_pool.tile([128, gpt, tps, N], f32)
                    # rowq[p, tp, :] = output row of sample (st, tp, p)
                    rowq = row_pool.tile([128, tps, W], f32)

                    for tp in range(tps):
                        t = st * tps + tp
                        rows = slice(t * PT, (t + 1) * PT)
                        # --- load + cast to fp16, natural layout [s, j, d] ---
                        nat = nat_pool.tile([128, N, D], f16)
                        if t % 2 == 0:
                            nc.gpsimd.dma_start(out=nat[:, 0, :], in_=dense[rows, :])
                            nc.gpsimd.dma_start(
                                out=nat[:, 1:N, :],
                                in_=sparse[:, rows, :].rearrange("s b d -> b s d"),
                            )
                        else:
                            nat32 = nat32_pool.tile([128, N, D], f32)
                            nc.sync.dma_start(out=nat32[:, 0, :], in_=dense[rows, :])
                            nc.sync.dma_start(
                                out=nat32[:, 1:N, :],
                                in_=sparse[:, rows, :].rearrange("s b d -> b s d"),
                            )
                            nc.scalar.copy(out=nat[:, :, :], in_=nat32[:, :, :])
                        # dense passthrough columns (f32, straight from HBM)
                        nc.sync.dma_start(out=rowq[:, tp, 0:D], in_=dense[rows, :])

                        # --- TensorE transpose of each feature slab ---
                        # 32 slabs: 27 features + 5 zero pad so Gram lhsT is M=32
                        xt = xt_pool.tile([128, 32, PT], f16)
                        nc.vector.memset(xt[:, N:32, :], 0.0)
                        for k in range(7):  # 4-slab packs: 6*4 + 3
                            j0 = 4 * k
                            nj = min(4, N - j0)
                            pt_ = psumt_pool.tile([128, 4, PT], f16, tag="pt")
                            for jj in range(nj):
                                nc.tensor.transpose(
                                    pt_[:, jj, :], nat[:, j0 + jj, :], id16
                                )
                            cp = nc.vector.tensor_copy if k % 2 else nc.scalar.copy
                            cp(out=xt[:, j0 : j0 + nj, :], in_=pt_[:, 0:nj, :])

                        # --- Gram matmuls ---
                        for r in range(rpt):
                            ps = psum_pool.tile([128, GPR, N], f32)
                            for q in range(GPR):
                                g_local = r * GPR + q
                                for c in range(4):
                                    s_local = 32 * c + g_local
                                    nc.tensor.matmul(
                                        out=ps[32 * c : 32 * c + 32, q, :],
                                        lhsT=xt[:, :, s_local],
                                        rhs=xt[:, 0:N, s_local],
                                        start=True,
                                        stop=True,
                                        tile_position=(0, 32 * c),
                                    )
                            off = r * GPR
                            nc.vector.tensor_copy(
                                out=gcol[:, off : off + GPR, tp, :],
                                in_=ps[:, :, :],
                            )

                    # --- flatten: Gram row i of sample (tp, 32c+g) from
                    # gcol[32c+i, g, tp, 0:i+1] to rowq[32c+g, tp, toff:] ---
                    for i in range(N):
                        toff = D + i * (i + 1) // 2
                        for c in range(4):
                            eng = nc.sync if (i + c) % 2 == 0 else nc.scalar
                            eng.dma_start(
                                # write iter (g->partition, tp, j)
                                out=rowq[32 * c : 32 * c + 32, :, toff : toff + i + 1],
                                # read iter (g, tp, j) on partition 32c+i
                                in_=gcol[32 * c + i : 32 * c + i + 1, :, :, 0 : i + 1],
                            )

                    # --- contiguous stores: 128 rows (= 259 KB) per tile ---
                    for tp in range(tps):
                        t = st * tps + tp
                        nc.scalar.dma_start(
                            out=out[t * PT : (t + 1) * PT, :],
                            in_=rowq[:, tp, :],
                        )

    nc.compile()
    return nc


_CACHE: dict = {}


def _get_nc():
    if "nc" not in _CACHE:
        _CACHE["nc"] = build_kernel(BC)
    return _CACHE["nc"]


def kernel(dense_feature, sparse_stack, **run_kwargs):
    dense_feature = np.asarray(dense_feature, dtype=np.float32)
    sparse_stack = np.asarray(sparse_stack, dtype=np.float32)
    assert dense_feature.shape == (B, D)
    assert sparse_stack.shape == (S, B, D)

    nc = run_kwargs.pop("nc", None) or _get_nc()
    in_maps = []
    for ci in range(NCORES):
        sl = slice(ci * BC, (ci + 1) * BC)
        in_maps.append(
            {
                "dense": np.ascontiguousarray(dense_feature[sl]),
                "sparse": np.ascontiguousarray(sparse_stack[:, sl, :]),
            }
        )
    res = bass_utils.run_bass_kernel_spmd(
        nc, in_maps, core_ids=list(range(NCORES)), **run_kwargs
    )
    out = np.concatenate([r["out"] for r in res.results], axis=0)
    if run_kwargs:
        _CACHE["last_result"] = res
    return out
